# revision 1
# baseline (speedup 1.0000x reference)
"""Trainium2 Bass kernel for Mixtral SwiGLU MLP with HQQ 4-bit weights.

Strategy:
  - Tensor-parallel over the intermediate dim (14336 -> 1792 per core, 8 cores).
  - Host dequantizes the HQQ weights to bf16 ((Q - z) * s, one rounding) and
    pre-permutes every tensor into the exact SBUF tile layout so all device
    DMAs are contiguous per partition (>= 8KB lines -> full HBM bandwidth).
  - Per 1024-token super-block: g/u projections as bf16 matmuls (f32 psum)
    with each weight tile reused for 2 token sub-blocks, SiLU on ScalarE,
    h = silu(g)*u on VectorE, then the down projection with each h tile
    reused for 2 output-column blocks. Partial outputs ([tok, hid] f32) are
    summed across cores on the host.
"""

import os
import sys

for _p in ("/opt/trn_rl_repo", "/root/.axon_site/_ro/trn_rl_repo"):
    if os.path.isdir(_p) and _p not in sys.path:
        sys.path.insert(0, _p)

import ml_dtypes
import numpy as np

import concourse.bacc as bacc
import concourse.mybir as mybir
import concourse.tile as tile
from concourse.bass_utils import run_bass_kernel_spmd

BF16 = ml_dtypes.bfloat16

N_CORES = 8
TOK = 4096
HID = 4096
INT = 14336
GS = 64

INT_SH = INT // N_CORES          # 1792 intermediate rows per core
TS = 1024                        # token super-block
SUPERS = TOK // TS               # 4
I_TILES = INT_SH // 128          # 14
H_TILES = HID // 128             # 32
DP_W = 1024                      # output-column pair width
DPS = HID // DP_W                # 4

_CACHE = {}


def _build_nc(repeats=1):
    key = ("nc", repeats)
    if key in _CACHE:
        return _CACHE[key]

    nc = bacc.Bacc("TRN2", target_bir_lowering=False, debug=False)
    bf = mybir.dt.bfloat16
    f32 = mybir.dt.float32

    # all inputs pre-permuted on host so DMAs are contiguous per partition
    x_d = nc.dram_tensor("xt", [SUPERS, 128, H_TILES, TS], bf, kind="ExternalInput")
    w1_d = nc.dram_tensor("w1t", [I_TILES, 128, H_TILES, 128], bf, kind="ExternalInput")
    w3_d = nc.dram_tensor("w3t", [I_TILES, 128, H_TILES, 128], bf, kind="ExternalInput")
    w2_d = nc.dram_tensor("w2t", [DPS, 128, I_TILES, DP_W], bf, kind="ExternalInput")
    out_d = nc.dram_tensor("out", [TOK, HID], f32, kind="ExternalOutput")

    Silu = mybir.ActivationFunctionType.Silu

    with tile.TileContext(nc) as tc:
        with (
            tc.tile_pool(name="xtp", bufs=1) as xtp,
            tc.tile_pool(name="w13p", bufs=2) as w13p,
            tc.tile_pool(name="hp", bufs=1) as hp,
            tc.tile_pool(name="w2p", bufs=2) as w2p,
            tc.tile_pool(name="op", bufs=3) as op,
            tc.tile_pool(name="tmpp", bufs=3) as tmpp,
            tc.tile_pool(name="psA", bufs=1, space="PSUM") as psA,
            tc.tile_pool(name="psB", bufs=2, space="PSUM") as psB,
        ):
            for sb in [s for _ in range(repeats) for s in range(SUPERS)]:
                xt_sb = xtp.tile([128, H_TILES, TS], bf, tag="xt")
                for q in range(8):
                    nc.sync.dma_start(
                        xt_sb[:, q * 4:(q + 1) * 4, :],
                        x_d[sb, :, q * 4:(q + 1) * 4, :],
                    )
                h_sb = hp.tile([128, I_TILES, TS], bf, tag="h")

                for it in range(I_TILES):
                    w1_sb = w13p.tile([128, H_TILES, 128], bf, tag="w1")
                    nc.sync.dma_start(w1_sb[:], w1_d[it])
                    w3_sb = w13p.tile([128, H_TILES, 128], bf, tag="w3")
                    nc.gpsimd.dma_start(w3_sb[:], w3_d[it])

                    g0 = psA.tile([128, 512], f32, tag="g0")
                    g1 = psA.tile([128, 512], f32, tag="g1")
                    u0 = psA.tile([128, 512], f32, tag="u0")
                    u1 = psA.tile([128, 512], f32, tag="u1")
                    for ht in range(H_TILES):
                        w = w1_sb[:, ht, :]
                        nc.tensor.matmul(g0[:], w, xt_sb[:, ht, 0:512],
                                         start=(ht == 0), stop=(ht == H_TILES - 1))
                        nc.tensor.matmul(g1[:], w, xt_sb[:, ht, 512:1024],
                                         start=(ht == 0), stop=(ht == H_TILES - 1))
                    for ht in range(H_TILES):
                        w = w3_sb[:, ht, :]
                        nc.tensor.matmul(u0[:], w, xt_sb[:, ht, 0:512],
                                         start=(ht == 0), stop=(ht == H_TILES - 1))
                        nc.tensor.matmul(u1[:], w, xt_sb[:, ht, 512:1024],
                                         start=(ht == 0), stop=(ht == H_TILES - 1))
                    for s, (g, u) in enumerate(((g0, u0), (g1, u1))):
                        sil = tmpp.tile([128, 512], bf, tag="sil")
                        nc.scalar.activation(sil[:], g[:], Silu)
                        nc.vector.tensor_mul(
                            h_sb[:, it, s * 512:(s + 1) * 512], sil[:], u[:])

                for dp in range(DPS):
                    w2_sb = w2p.tile([128, I_TILES, DP_W], bf, tag="w2")
                    nc.sync.dma_start(
                        w2_sb[:, 0:7, :], w2_d[dp, :, 0:7, :])
                    nc.sync.dma_start(
                        w2_sb[:, 7:I_TILES, :], w2_d[dp, :, 7:I_TILES, :])
                    for tt in range(TS // 128):
                        o0 = psB.tile([128, 512], f32, tag="o0")
                        o1 = psB.tile([128, 512], f32, tag="o1")
                        for it in range(I_TILES):
                            h_t = h_sb[:, it, tt * 128:(tt + 1) * 128]
                            nc.tensor.matmul(o0[:], h_t, w2_sb[:, it, 0:512],
                                             start=(it == 0),
                                             stop=(it == I_TILES - 1))
                            nc.tensor.matmul(o1[:], h_t, w2_sb[:, it, 512:1024],
                                             start=(it == 0),
                                             stop=(it == I_TILES - 1))
                        rows = slice(sb * TS + tt * 128, sb * TS + (tt + 1) * 128)
                        for s, o_ps in ((0, o0), (1, o1)):
                            o_sb = op.tile([128, 512], f32, tag="osb")
                            nc.vector.tensor_copy(o_sb[:], o_ps[:])
                            cols = slice(dp * DP_W + s * 512,
                                         dp * DP_W + (s + 1) * 512)
                            nc.gpsimd.dma_start(out_d[rows, cols], o_sb[:])

    nc.compile()
    _CACHE[key] = nc
    return nc


def _dequant(q, s, z):
    """(Q - z) * s with per-group broadcast; returns f32 [out, in]."""
    out, inp = q.shape
    g = inp // GS
    qf = np.asarray(q, np.float32).reshape(out, g, GS)
    w = (qf - np.asarray(z, np.float32)[:, :, None]) * \
        np.asarray(s, np.float32)[:, :, None]
    return w.reshape(out, inp)


def _prep_in_maps(hidden_states, w1_q, w1_scale, w1_zero, w3_q, w3_scale,
                  w3_zero, w2_q, w2_scale, w2_zero):
    x = np.asarray(hidden_states, np.float32)

    # xt[sb, p, a, t] = x[sb*TS + t, a*128 + p]
    xt = np.ascontiguousarray(
        x.astype(BF16).reshape(SUPERS, TS, H_TILES, 128).transpose(0, 3, 2, 1)
    )

    def up_shard(q, s, z, c):
        rows = slice(c * INT_SH, (c + 1) * INT_SH)
        wd = _dequant(q[rows], s[rows], z[rows]).astype(BF16)  # [INT_SH, HID]
        # w1t[it, p, a, i] = wd[it*128 + i, a*128 + p]
        return np.ascontiguousarray(
            wd.reshape(I_TILES, 128, H_TILES, 128).transpose(0, 3, 2, 1)
        )

    def down_shard(q, s, z, c):
        cols = slice(c * INT_SH, (c + 1) * INT_SH)
        gsl = slice(c * (INT_SH // GS), (c + 1) * (INT_SH // GS))
        wd = _dequant(np.ascontiguousarray(q[:, cols]), s[:, gsl],
                      z[:, gsl]).astype(BF16)                   # [HID, INT_SH]
        # w2t[dp, p, a, d] = wd[dp*DP_W + d, a*128 + p]
        return np.ascontiguousarray(
            wd.reshape(DPS, DP_W, I_TILES, 128).transpose(0, 3, 2, 1)
        )

    in_maps = []
    for c in range(N_CORES):
        in_maps.append({
            "xt": xt,
            "w1t": up_shard(w1_q, w1_scale, w1_zero, c),
            "w3t": up_shard(w3_q, w3_scale, w3_zero, c),
            "w2t": down_shard(w2_q, w2_scale, w2_zero, c),
        })
    return in_maps


def kernel(**inputs):
    nc = _build_nc()
    in_maps = _prep_in_maps(**inputs)
    res = run_bass_kernel_spmd(nc, in_maps, core_ids=list(range(N_CORES)))
    out = np.zeros((TOK, HID), np.float64)
    for c in range(N_CORES):
        out += res.results[c]["out"].astype(np.float64)
    return out.astype(np.float32)


if __name__ == "__main__":
    rng = np.random.default_rng(0)
    ins = {
        "hidden_states": rng.standard_normal((TOK, HID)).astype(np.float32),
        "w1_q": rng.integers(0, 16, (INT, HID)).astype(np.int32),
        "w1_scale": rng.random((INT, HID // GS)).astype(np.float32),
        "w1_zero": rng.random((INT, HID // GS)).astype(np.float32),
        "w3_q": rng.integers(0, 16, (INT, HID)).astype(np.int32),
        "w3_scale": rng.random((INT, HID // GS)).astype(np.float32),
        "w3_zero": rng.random((INT, HID // GS)).astype(np.float32),
        "w2_q": rng.integers(0, 16, (HID, INT)).astype(np.int32),
        "w2_scale": rng.random((HID, INT // GS)).astype(np.float32),
        "w2_zero": rng.random((HID, INT // GS)).astype(np.float32),
    }
    out = kernel(**ins)
    print("out", out.shape, out.dtype, float(np.abs(out).max()))

